# revision 21
# baseline (speedup 1.0000x reference)
"""Trainium2 Bass kernel for nn_Compress (ragged window compression).

Computation per window w (W=6128 windows, K=32 tokens each):
  toks = buffer[window_token_idx[w]]                  # [K, H, D]
  gates = sigmoid(RESCALE * flat(toks) @ gate_weight.T)   # [H, K]
  out[w] = einsum('hk,khd->hd', gates, toks)          # [H, D]
  compressed_buffer[dest_idx[w]] = out[w]

Sharding (per the data-parallel-by-request hint): 766 windows/core (padded
to 768); each core owns the slice of the kv buffer its windows reference —
the host compacts those rows (<=24576 of them) into a per-core fp16 table,
which also makes the gather indices fit int16 for the production dma_gather
path.  gate_weight is replicated.

Per-core device pipeline (3 groups x 256 windows):
  1. dma_gather: 8192 token rows (512B each) -> Xb [128=(w_l,k'), t, (h,d)] f16
  2. xbar DMA-transpose -> U [d=128, (t,h), (w_l,k')] f16
  3. PE: gate logits = GWt[k'].T @ U chunks (accumulate over k'), psum f32
  4. ACT: sigmoid(logits * RESCALE)
  5. PE: selection matmul + DVE mask -> block-diagonal gates Gblk
  6. PE: wsum out[d, w] = Xb[:, t, h, :].T @ Gblk[:, h, t, :]
  7. ACT copy psum->SBUF, DMA to DRAM out [d, (w,h)] (host decodes)

Every DMA instruction sits on an explicit total order (nop-funnelled single
waits) because the DMA ISA structs have a tiny sync-wait budget and the
xbar transpose serializes against in-flight DMAs anyway.
"""

import sys

sys.path.insert(0, "/opt/trn_rl_repo")

import numpy as np

import concourse.bass as bass
import concourse.mybir as mybir
import concourse.tile as tile
from concourse import bacc
from concourse.bass_utils import run_bass_kernel_spmd

# Problem constants (hardcoded per harness contract).
BUF = 200000
CBUF = 40000
H = 2
D = 128
K = 32  # window kernel size == gate count
NCORES = 8
TPG = 64  # tiles (of 4 windows) per group
GROUPS = 3
TILES = GROUPS * TPG  # 192 tiles -> 768 windows (2 padding)
WPAD = 4 * TILES
UMAX = WPAD * K  # compacted per-core buffer rows (worst case all unique)
NPG = TPG * 128  # tokens gathered per group
RESCALE = float((K * D) ** -0.5)

F32 = mybir.dt.float32
F16 = mybir.dt.float16
I16 = mybir.dt.int16

NP16 = np.float16


def build_nc():
    nc = bacc.Bacc("TRN2", target_bir_lowering=False, debug=False)

    bufc = nc.dram_tensor("bufc", [UMAX, H * D], F16, kind="ExternalInput")
    wti16 = nc.dram_tensor("wti16", [128, TILES * 8], I16, kind="ExternalInput")
    gwt = nc.dram_tensor("gwt", [128, K, K], F16, kind="ExternalInput")
    sel2 = nc.dram_tensor("sel2", [K, 128], F32, kind="ExternalInput")
    maskd = nc.dram_tensor("maskd", [128, H * TPG * 4], F32, kind="ExternalInput")
    outd = nc.dram_tensor("outd", [128, WPAD * H], F32, kind="ExternalOutput")

    from concourse.tile_rust import add_dep_helper

    def funnel(eng, deps):
        # One nop per dependency: each nop carries exactly one sync wait, so
        # the guarded DMA (tiny ISA wait-slot budget) needs none of them.
        nops = []
        for dep in deps:
            if dep is None:
                continue
            n = eng.nop(nofuse=True)
            add_dep_helper(n.ins, dep, reason="wait funnel")
            nops.append(n.ins)
        return nops

    def guard(inst, nops):
        for n in nops:
            add_dep_helper(inst, n, sync=True, reason="wait funnel order")
        return inst

    with tile.TileContext(nc) as tc:
        with (
            tc.tile_pool(name="const", bufs=1) as cpool,
            tc.tile_pool(name="data", bufs=2) as dpool,
            tc.tile_pool(name="psum", bufs=2, space="PSUM") as ppool,
        ):
            # dma_gather is a GPSIMD ucode extended instruction; its library
            # must be loaded before the first use.
            from concourse import library_config

            libload = nc.gpsimd.load_library(library_config.mlp)

            # Explicit total order over every DMA instruction.
            dma_chain = []

            gwt_sb = cpool.tile([128, K, K], F16)
            dma_chain.append(
                nc.sync.dma_start(out=gwt_sb[:, :, :], in_=gwt[:, :, :]).ins
            )
            sel2_sb = cpool.tile([K, 128], F32)
            dma_chain.append(
                guard(
                    nc.sync.dma_start(out=sel2_sb[:, :], in_=sel2[:, :]).ins,
                    funnel(nc.sync, [dma_chain[-1]]),
                )
            )
            mask_sb = cpool.tile([128, H * TPG * 4], F32)
            dma_chain.append(
                guard(
                    nc.sync.dma_start(out=mask_sb[:, :], in_=maskd[:, :]).ins,
                    funnel(nc.sync, [dma_chain[-1]]),
                )
            )
            idx_sb = cpool.tile([128, TILES * 8], I16)
            dma_chain.append(
                guard(
                    nc.sync.dma_start(out=idx_sb[:, :], in_=wti16[:, :]).ins,
                    funnel(nc.sync, [dma_chain[-1]]),
                )
            )

            gather_inst = [None] * GROUPS
            tp_inst = [None] * GROUPS
            last_logits_mm = [None] * GROUPS
            last_wsum_mm = [None] * GROUPS

            for g in range(GROUPS):
                # 1. gather: Xb[p=(w_l*32+k'), t, h*128+d]
                Xb = dpool.tile([128, TPG, H * D], F16, tag="Xb")
                gnops = funnel(
                    nc.gpsimd,
                    [
                        dma_chain[-1],
                        last_wsum_mm[g - 2] if g >= 2 else None,
                    ],
                )
                if g == 0:
                    gnops.append(libload.ins)
                # 2048-idx multi-packet sub-gathers (HW-validated size).
                SUB = 2048
                nsub = NPG // SUB
                tps = SUB // 128  # tiles per sub-gather
                for j in range(nsub):
                    gi = nc.gpsimd.dma_gather(
                        out_ap=Xb[:, j * tps : (j + 1) * tps, :],
                        in_ap=bufc[:, :],
                        idxs_ap=idx_sb[
                            :,
                            g * (NPG // 16) + j * (SUB // 16) : g * (NPG // 16)
                            + (j + 1) * (SUB // 16),
                        ],
                        num_idxs=SUB,
                        num_idxs_reg=SUB,
                        elem_size=H * D,
                        single_packet=False,
                    )
                    if j == 0:
                        guard(gi.ins, gnops)
                    dma_chain.append(gi.ins)
                gather_inst[g] = gi.ins

                # 2. xbar transpose: U[d, (t,h), (w_l,k')]
                U = dpool.tile([128, TPG * H, 128], F16, tag="U")
                tnops = funnel(
                    nc.sync,
                    [
                        dma_chain[-1],
                        last_logits_mm[g - 2] if g >= 2 else None,
                    ],
                )
                tp = nc.sync.dma_start(
                    out=U[:, :, :],
                    in_=Xb[:, :, :].rearrange("p t f -> p (t f)"),
                    transpose=True,
                )
                tp_inst[g] = guard(tp.ins, tnops)
                dma_chain.append(tp.ins)
                U5 = U[:, :, :].rearrange(
                    "d (t h) (w k) -> d t h w k", t=TPG, h=H, w=4, k=K
                )

                # 3. gate logits: accumulate over k' chunks
                plg = ppool.tile([K, H, TPG * 4], F32, tag="plg")
                for h in range(H):
                    for kp in range(K):
                        mm = nc.tensor.matmul(
                            plg[:, h, :],
                            lhsT=gwt_sb[:, kp, :],
                            rhs=U5[:, :, h, :, kp],
                            start=(kp == 0),
                            stop=(kp == K - 1),
                        )
                last_logits_mm[g] = mm.ins

                # 4. gates = sigmoid(logits * RESCALE)   [K, (h,t,w)]
                gates = dpool.tile([K, H * TPG * 4], F32, tag="gates")
                nc.scalar.activation(
                    gates[:, :],
                    plg[:, :, :],
                    mybir.ActivationFunctionType.Sigmoid,
                    scale=RESCALE,
                )

                # 5. block-diagonal gates: Gblk[(w_l,k'), (h,t,w)]
                pG = ppool.tile([128, H * TPG * 4], F32, tag="pG")
                nc.tensor.matmul(
                    pG[:, :],
                    lhsT=sel2_sb[:, :],
                    rhs=gates[:, :],
                    start=True,
                    stop=True,
                )
                Gblk = dpool.tile([128, H * TPG * 4], F16, tag="Gblk")
                nc.vector.tensor_tensor(
                    out=Gblk[:, :],
                    in0=pG[:, :],
                    in1=mask_sb[:, :],
                    op=mybir.AluOpType.mult,
                )
                Gblk4 = Gblk[:, :].rearrange(
                    "p (h t w) -> p h t w", h=H, t=TPG, w=4
                )

                # 6. wsum: out[d, w] = sum_k' gates * toks
                pw = ppool.tile([128, TPG, H, 4], F32, tag="pw")
                for t in range(TPG):
                    for h in range(H):
                        mm = nc.tensor.matmul(
                            pw[:, t, h, :],
                            lhsT=Xb[:, t, h * D : (h + 1) * D],
                            rhs=Gblk4[:, h, t, :],
                            start=True,
                            stop=True,
                        )
                last_wsum_mm[g] = mm.ins

                # 7. copy out + DMA (cols = (t, h, w))
                stage = dpool.tile([128, TPG * 4 * H], F32, tag="stage")
                stage_copy = nc.scalar.activation(
                    stage[:, :],
                    pw[:, :, :, :],
                    mybir.ActivationFunctionType.Copy,
                )
                snops = funnel(nc.sync, [dma_chain[-1], stage_copy.ins])
                dma_chain.append(
                    guard(
                        nc.sync.dma_start(
                            out=outd[
                                :, g * (TPG * 4 * H) : (g + 1) * (TPG * 4 * H)
                            ],
                            in_=stage[:, :],
                        ).ins,
                        snops,
                    )
                )
    nc.finalize()
    return nc


def core_span(n_windows, core):
    wc = -(-n_windows // NCORES)  # ceil
    assert wc <= WPAD, (n_windows, wc)
    w0 = core * wc
    return w0, min(wc, max(0, n_windows - w0))


def prep_core_inputs(buffer16, wti_full, gwt16, sel2, maskd, core):
    """Host-side shard prep: compact this core's buffer rows + int16 idx."""
    w0, nw = core_span(wti_full.shape[0], core)
    wti_core = np.zeros((WPAD, K), dtype=np.int64)
    wti_core[:nw] = wti_full[w0 : w0 + nw]

    uniq, inv = np.unique(wti_core.reshape(-1), return_inverse=True)
    assert len(uniq) <= UMAX
    bufc = np.zeros((UMAX, H * D), dtype=NP16)
    bufc[: len(uniq)] = buffer16[uniq]
    inv = inv.reshape(WPAD, K)

    # gather order i = t*128 + p  (p = w_l*32 + k', tile T = g*TPG + t)
    p = np.arange(128)
    TT = np.arange(TILES)
    I = inv[TT[:, None] * 4 + (p // K)[None, :], (p % K)[None, :]]  # [TILES,128]
    flat = I.reshape(GROUPS, NPG)  # per group, i = t*128+p
    # 16-partition wrap, replicated to 128 partitions
    s = np.arange(NPG // 16)
    wrapped = flat[:, s[None, :] * 16 + (p % 16)[:, None]]  # [G, 128, NPG/16]
    idx16 = np.ascontiguousarray(
        np.transpose(wrapped, (1, 0, 2)).reshape(128, TILES * 8)
    ).astype(np.int16)

    return {
        "bufc": bufc,
        "wti16": idx16,
        "gwt": gwt16,
        "sel2": sel2,
        "maskd": maskd,
    }


def prep_all_inputs(buffer, gate_weight, window_token_idx):
    buffer16 = (
        np.asarray(buffer, dtype=np.float32).reshape(BUF, H * D).astype(NP16)
    )
    gw = np.asarray(gate_weight, dtype=np.float32)
    # gwt[d, k', k] = gw[k, k'*128 + d]
    gwt16 = np.ascontiguousarray(
        gw.reshape(K, K, D).transpose(2, 1, 0).astype(NP16)
    )
    sel2 = np.ascontiguousarray(np.tile(np.eye(K, dtype=np.float32), (1, 4)))
    cols = np.arange(H * TPG * 4)
    maskd = ((cols % 4)[None, :] == (np.arange(128) // K)[:, None]).astype(
        np.float32
    )
    wti_full = np.asarray(window_token_idx, dtype=np.int64)
    return [
        prep_core_inputs(buffer16, wti_full, gwt16, sel2, maskd, c)
        for c in range(NCORES)
    ]


def decode_outd(dev):
    """Device out [128, WPAD*H] with cols (g,t,h,w) -> [WPAD, H, D]."""
    arr = np.asarray(dev, dtype=np.float32).reshape(128, TILES, H, 4)
    return np.ascontiguousarray(
        arr.transpose(1, 3, 2, 0).reshape(WPAD, H, D)
    )


def assemble_output(results, compressed_buffer, dest_idx):
    dest = np.asarray(dest_idx, dtype=np.int64)
    nwin = dest.shape[0]
    out = np.array(np.asarray(compressed_buffer, dtype=np.float32), copy=True)
    for c in range(NCORES):
        w0, nw = core_span(nwin, c)
        if nw <= 0:
            continue
        rows = decode_outd(results[c]["outd"])[:nw]
        out[dest[w0 : w0 + nw]] = rows
    return out


_NC_CACHE = None


def _get_nc():
    global _NC_CACHE
    if _NC_CACHE is None:
        _NC_CACHE = build_nc()
    return _NC_CACHE


def run_cores(in_maps, trace=False, trace_kwargs=None):
    nc = _get_nc()
    res = run_bass_kernel_spmd(
        nc,
        in_maps,
        list(range(NCORES)),
        trace=trace,
        **(trace_kwargs or {}),
    )
    return res


def kernel(
    buffer, compressed_buffer, gate_weight, window_token_idx, dest_idx
):
    in_maps = prep_all_inputs(buffer, gate_weight, window_token_idx)
    res = run_cores(in_maps)
    return assemble_output(res.results, compressed_buffer, dest_idx)
